# revision 32
# baseline (speedup 1.0000x reference)
"""BatchSiren Trainium2 kernel.

B=2048 independent SIREN MLPs (2->32->32->3, sin activations, w0=30),
each evaluated on the same N=1024 coordinate grid.

Strategy (pure data parallel over 8 cores, 256 nets/core):
- 16 supergroups of 16 nets per core; nets packed 16-at-a-time onto the
  128x128 PE array via 32x32 tile_position.
- All matmuls run single-pass (bf16/fp16, 1 cyc/row). Layer 1 uses an
  exact bf16 hi/lo split of coords+weights (K=8) so its products are
  exact in fp32 PSUM accumulation; layers 2/3 use fp16 weights/acts.
- sin(w0*z): weights pre-scaled by w0/2pi so PSUM holds the angle in
  CYCLE units t. Range reduction: DVE copies PSUM+bias+192 to bf16
  (bf16 RNE at ulp=1 yields round(t)+192 exactly), a -I matmul
  accumulates it back into PSUM leaving frac(t)-192, and ACT Sin reads
  PSUM with scale=+2pi, bias=384pi+..., writing fp16 H to SBUF.
- Output: one [128,2048] fp32 tile per (4sg, half), 4 batched DMAs each.
"""
import numpy as np
import ml_dtypes

import concourse.bacc as bacc
import concourse.bass as bass
import concourse.mybir as mybir
import concourse.tile as tile
from concourse import bass_utils

f32 = mybir.dt.float32
f16 = mybir.dt.float16
bf16 = mybir.dt.bfloat16
AF = mybir.ActivationFunctionType
ALU = mybir.AluOpType

W0 = 30.0
PI = float(np.pi)
TWO_PI = float(2 * np.pi)
MAG = 192.0                      # bf16 magic: exact ints in [176, 208]
BIAS1 = float(384.0 * np.pi)     # 2*pi*MAG
SD_L1 = []        # L1 banks whose subtract runs on DVE (else PE -I)
SD_L2 = [2, 3]    # L2 banks whose subtract runs on DVE
N_CORES = 8
B, N, IN, H, OUT = 2048, 1024, 2, 32, 3
BPC = B // N_CORES        # 256 nets per core
SGS = BPC // 16           # 16 supergroups of 16 nets
NH = N // 2               # 512 points per half
GRP = 4                   # supergroups per output tile

_compiled = None


def _build_module():
    nc = bacc.Bacc("TRN2", target_bir_lowering=False, debug=False)

    d_w1 = nc.dram_tensor("w1s", [4, 8, 128 * SGS], bf16, kind="ExternalInput")
    d_w2 = nc.dram_tensor("w2s", [4, 32, 128 * SGS], f16, kind="ExternalInput")
    d_w3 = nc.dram_tensor("w3blk", [4, 32, 48 * SGS], f16, kind="ExternalInput")
    d_sm = nc.dram_tensor("smalls", [128, 11 * SGS], f32, kind="ExternalInput")
    d_c = nc.dram_tensor("coords", [4, 8, N], bf16, kind="ExternalInput")
    d_ni = nc.dram_tensor("negI", [128, 32], bf16, kind="ExternalInput")
    d_out = nc.dram_tensor("out", [2, 48, SGS, NH], f32, kind="ExternalOutput")

    with tile.TileContext(nc, pool_alloc_mode="queue") as tc:
        with tc.tile_pool(name="const", bufs=1) as cp, \
             tc.tile_pool(name="acts", bufs=2) as ap, \
             tc.tile_pool(name="outp", bufs=4) as op_, \
             tc.tile_pool(name="ps", bufs=4, space="PSUM") as psp:

            # ---- persistent constants (sg0-critical tensors first) ----
            w1sb = cp.tile([128, 128 * SGS], bf16, tag="w1")
            for a in range(4):
                nc.sync.dma_start(w1sb[32 * a:32 * a + 8, :], d_w1[a])
            c4 = cp.tile([128, N], bf16, tag="c4")
            nc.vector.memset(c4[:], 0.0)  # pad rows 8-31 of each group
            for a in range(4):
                nc.sync.dma_start(c4[32 * a:32 * a + 8, :], d_c[a])
            negI = cp.tile([128, 32], bf16, tag="ni")
            nc.sync.dma_start(negI[:], d_ni[:])
            smalls = cp.tile([128, 11 * SGS], f32, tag="sm")
            nc.sync.dma_start(smalls[:], d_sm[:])
            w2sb = cp.tile([128, 128 * SGS], f16, tag="w2")
            q = 128 * SGS // 4
            for ck in range(4):
                for b in range(4):
                    nc.sync.dma_start(
                        w2sb[32 * b:32 * b + 32, q * ck:q * ck + q],
                        d_w2[b, :, q * ck:q * ck + q])
            w3sb = cp.tile([128, 48 * SGS], f16, tag="w3")
            for a in range(4):
                nc.sync.dma_start(w3sb[32 * a:32 * a + 32, :], d_w3[a])
            bias1 = cp.tile([128, 1], f32, tag="b1")
            nc.vector.memset(bias1[:], BIAS1)
            mag192 = cp.tile([128, 1], f32, tag="mg")
            nc.vector.memset(mag192[:], MAG)

            # ---- per-(sg,h) stages ----
            def stage_A(sg, h):       # L1 matmuls -> PZ1 (4 banks)
                PZ1 = [psp.tile([128, 512], f32, tag="P1",
                                name=f"pz1_{sg}_{h}_{a}") for a in range(4)]
                for a in range(4):
                    nc.tensor.matmul(
                        out=PZ1[a][:, :],
                        lhsT=w1sb[32 * a:32 * a + 8,
                                  128 * sg:128 * sg + 128],
                        rhs=c4[32 * a:32 * a + 8, NH * h:NH * h + NH],
                        start=True, stop=(a in SD_L1),
                        tile_position=(32 * a, 0))
                return PZ1

            def stage_E1(sg, h, PZ1):  # round-evac L1 -> R1 bf16
                R1 = ap.tile([128, 2048], bf16, tag="R")
                for a in range(4):
                    nc.vector.tensor_scalar(
                        R1[:, 512 * a:512 * a + 512], PZ1[a][:],
                        MAG, 0.0, ALU.add, ALU.add)
                return R1

            def stage_S1(sg, h, PZ1, R1):  # -I accumulate (PE banks only)
                for a in range(4):
                    if a in SD_L1:
                        continue
                    for b in range(4):
                        nc.tensor.matmul(
                            out=PZ1[a][32 * b:32 * b + 32, :],
                            lhsT=negI[32 * b:32 * b + 32, :],
                            rhs=R1[32 * b:32 * b + 32, 512 * a:512 * a + 512],
                            start=False, stop=True,
                            tile_position=(32 * b, 32 * b))
                if not SD_L1:
                    return None
                Q1 = ap.tile([128, 512 * len(SD_L1)], f32, tag="Q1")
                for i, a in enumerate(SD_L1):
                    nc.vector.scalar_tensor_tensor(
                        Q1[:, 512 * i:512 * i + 512],
                        R1[:, 512 * a:512 * a + 512], 0.0, PZ1[a][:],
                        ALU.add, ALU.subtract)
                return Q1

            def stage_N1(sg, h, PZ1, Q1):  # Sin -> H1 fp16
                H1 = ap.tile([128, 2048], f16, tag="H1")
                for a in range(4):
                    if a not in SD_L1:
                        nc.scalar.activation(
                            H1[:, 512 * a:512 * a + 512], PZ1[a][:],
                            AF.Sin, bias=bias1[:], scale=TWO_PI)
                for i, a in enumerate(SD_L1):
                    nc.scalar.activation(
                        H1[:, 512 * a:512 * a + 512],
                        Q1[:, 512 * i:512 * i + 512],
                        AF.Sin, bias=bias1[:], scale=-TWO_PI)
                return H1

            def stage_B(sg, h, H1):    # L2 matmuls -> PZ2 (4 banks)
                PZ2 = [psp.tile([128, 512], f32, tag="P2",
                                name=f"pz2_{sg}_{h}_{bt}") for bt in range(4)]
                for bt in range(4):
                    for a in range(4):
                        nc.tensor.matmul(
                            out=PZ2[bt][32 * a:32 * a + 32, :],
                            lhsT=w2sb[32 * bt:32 * bt + 32,
                                      128 * sg + 32 * a:128 * sg + 32 * a + 32],
                            rhs=H1[32 * bt:32 * bt + 32,
                                   512 * a:512 * a + 512],
                            start=True, stop=(bt in SD_L2),
                            tile_position=(32 * bt, 32 * a))
                return PZ2

            def stage_E2(sg, h, PZ2):  # round-evac L2 (+b2) -> R2 bf16
                R2 = ap.tile([128, 2048], bf16, tag="R")
                nc.scalar.activation(
                    R2[:, 0:512], PZ2[0][:], AF.Identity,
                    bias=smalls[:, 11 * sg:11 * sg + 1], scale=1.0)
                for bt in range(1, 4):
                    nc.vector.tensor_scalar(
                        R2[:, 512 * bt:512 * bt + 512], PZ2[bt][:],
                        smalls[:, 11 * sg + bt:11 * sg + bt + 1], 0.0,
                        ALU.add, ALU.add)
                return R2

            def stage_S2(sg, h, PZ2, R2):
                for bt in range(4):
                    if bt in SD_L2:
                        continue
                    for a in range(4):
                        nc.tensor.matmul(
                            out=PZ2[bt][32 * a:32 * a + 32, :],
                            lhsT=negI[32 * a:32 * a + 32, :],
                            rhs=R2[32 * a:32 * a + 32, 512 * bt:512 * bt + 512],
                            start=False, stop=True,
                            tile_position=(32 * a, 32 * a))
                if not SD_L2:
                    return None
                Q2 = ap.tile([128, 512 * len(SD_L2)], f32, tag="Q2")
                for i, bt in enumerate(SD_L2):
                    nc.vector.scalar_tensor_tensor(
                        Q2[:, 512 * i:512 * i + 512],
                        R2[:, 512 * bt:512 * bt + 512],
                        smalls[:, 11 * sg + 9 + i:11 * sg + 9 + i + 1],
                        PZ2[bt][:], ALU.add, ALU.subtract)
                return Q2

            def stage_N2(sg, h, PZ2, Q2):  # Sin (+b2 bias) -> H2 fp16
                H2 = ap.tile([128, 2048], f16, tag="H2")
                for bt in range(4):
                    if bt not in SD_L2:
                        nc.scalar.activation(
                            H2[:, 512 * bt:512 * bt + 512], PZ2[bt][:],
                            AF.Sin,
                            bias=smalls[:, 11 * sg + 4 + bt:11 * sg + 4 + bt + 1],
                            scale=TWO_PI)
                if False:
                    nc.scalar.activation(
                        H2[:, 1024:2048], Q2[:, 0:1024],
                        AF.Sin, bias=bias1[:], scale=-TWO_PI)
                else:
                    for i, bt in enumerate(SD_L2):
                        nc.scalar.activation(
                            H2[:, 512 * bt:512 * bt + 512],
                            Q2[:, 512 * i:512 * i + 512],
                            AF.Sin, bias=bias1[:], scale=-TWO_PI)
                return H2

            def stage_C(sg, h, H2):    # L3 matmuls -> PC (1 bank)
                PC = psp.tile([128, 512], f32, tag="P1",
                              name=f"pc_{sg}_{h}")
                for bt in range(4):
                    nc.tensor.matmul(
                        out=PC[32 * bt:32 * bt + 12, :],
                        lhsT=w3sb[:, 48 * sg + 12 * bt:48 * sg + 12 * bt + 12],
                        rhs=H2[:, 512 * bt:512 * bt + 512],
                        start=True, stop=True,
                        tile_position=(0, 32 * bt))
                return PC

            def stage_E3(sg, h, PC, OT):  # +b3, evac to output tile
                sgl = sg % GRP
                dst = OT[:, 512 * sgl:512 * sgl + 512]
                bias = smalls[:, 11 * sg + 8:11 * sg + 9]
                nc.vector.tensor_scalar(dst, PC[:], bias, 0.0,
                                        ALU.add, ALU.add)

            def flush_out(g, h, OT):
                for bt in range(4):
                    nc.sync.dma_start(
                        d_out[h, 12 * bt:12 * bt + 12, GRP * g:GRP * g + GRP, :],
                        OT[32 * bt:32 * bt + 12, :])

            # ---- software-pipelined emission ----
            # V: [E1x4, E2c1..3, E3]  S: [E2c0, N1x4, N2x4, E3]
            # T: [A, S1, S2, B, C]
            OTs = {}

            def get_OT(sg, h):
                g = sg // GRP
                if (g, h) not in OTs:
                    OTs[(g, h)] = op_.tile([128, 512 * GRP], f32, tag="OT",
                                           name=f"ot_{g}_{h}")
                return OTs[(g, h)]

            prev = None
            for sg in range(SGS):
                for h in range(2):
                    PZ1 = stage_A(sg, h)
                    if prev is not None:
                        psg, ph, pPZ2 = prev
                        R2 = ap.tile([128, 2048], bf16, tag="R",
                                     name=f"r2_{psg}_{ph}")
                        nc.scalar.activation(
                            R2[:, 0:512], pPZ2[0][:], AF.Identity,
                            bias=smalls[:, 11 * psg:11 * psg + 1], scale=1.0)
                        nc.scalar.activation(
                            R2[:, 512:1024], pPZ2[1][:], AF.Identity,
                            bias=smalls[:, 11 * psg + 1:11 * psg + 2],
                            scale=1.0)
                        # S2 banks 0,1 -I early: fill PE gap during E1
                        for bt0 in range(2):
                            for a2 in range(4):
                                nc.tensor.matmul(
                                    out=pPZ2[bt0][32 * a2:32 * a2 + 32, :],
                                    lhsT=negI[32 * a2:32 * a2 + 32, :],
                                    rhs=R2[32 * a2:32 * a2 + 32,
                                           512 * bt0:512 * bt0 + 512],
                                    start=False, stop=True,
                                    tile_position=(32 * a2, 32 * a2))
                    R1 = stage_E1(sg, h, PZ1)
                    Q1 = stage_S1(sg, h, PZ1, R1)
                    H1 = stage_N1(sg, h, PZ1, Q1)
                    if prev is not None:
                        Q2p = ap.tile([128, 512 * len(SD_L2)], f32, tag="Q2",
                                      name=f"q2_{psg}_{ph}")
                        for bt in range(2, 4):
                            nc.vector.tensor_scalar(
                                R2[:, 512 * bt:512 * bt + 512], pPZ2[bt][:],
                                smalls[:, 11 * psg + bt:11 * psg + bt + 1],
                                0.0, ALU.add, ALU.add)
                        for bt in range(2, 4):
                            if bt in SD_L2:
                                i = SD_L2.index(bt)
                                nc.vector.scalar_tensor_tensor(
                                    Q2p[:, 512 * i:512 * i + 512],
                                    R2[:, 512 * bt:512 * bt + 512],
                                    smalls[:, 11 * psg + 9 + i:11 * psg + 9 + i + 1],
                                    pPZ2[bt][:], ALU.add, ALU.subtract)
                                continue
                            for a2 in range(4):
                                nc.tensor.matmul(
                                    out=pPZ2[bt][32 * a2:32 * a2 + 32, :],
                                    lhsT=negI[32 * a2:32 * a2 + 32, :],
                                    rhs=R2[32 * a2:32 * a2 + 32,
                                           512 * bt:512 * bt + 512],
                                    start=False, stop=True,
                                    tile_position=(32 * a2, 32 * a2))
                        H2 = stage_N2(psg, ph, pPZ2, Q2p)
                    PZ2 = stage_B(sg, h, H1)
                    if prev is not None:
                        PC = stage_C(psg, ph, H2)
                        OT = get_OT(psg, ph)
                        stage_E3(psg, ph, PC, OT)
                        if (psg % GRP) == GRP - 1:
                            flush_out(psg // GRP, ph, OT)
                    prev = (sg, h, PZ2)
            # epilogue
            psg, ph, pPZ2 = prev
            R2 = stage_E2(psg, ph, pPZ2)
            Q2 = stage_S2(psg, ph, pPZ2, R2)
            H2 = stage_N2(psg, ph, pPZ2, Q2)
            PC = stage_C(psg, ph, H2)
            OT = get_OT(psg, ph)
            stage_E3(psg, ph, PC, OT)
            flush_out(psg // GRP, ph, OT)

    nc.compile()
    return nc


def _prep_core_inputs(w1, b1, w2, b2, w3, b3, coords, core):
    s = np.float32(W0 / TWO_PI)
    B0 = core * BPC
    sl = slice(B0, B0 + BPC)
    bf = ml_dtypes.bfloat16

    # ---- L1: bf16 hi/lo split, K=8 ----
    # net (sg, a, b) = batch B0 + 16sg + 4a + b
    w1c = (w1[sl, :, :, 0] * s).astype(np.float32).reshape(SGS, 4, 4, IN, H)
    b1c = (b1[sl, :, 0] * s).astype(np.float32).reshape(SGS, 4, 4, H)
    w1h = w1c.astype(bf)
    w1l = (w1c - w1h.astype(np.float32)).astype(bf)
    b1h = b1c.astype(bf)
    b1l = (b1c - b1h.astype(np.float32)).astype(bf)
    # rows: [wh0, wh1, wh0, wh1, wl0, wl1, bh, bl]
    rows = np.stack([w1h[:, :, :, 0], w1h[:, :, :, 1],
                     w1h[:, :, :, 0], w1h[:, :, :, 1],
                     w1l[:, :, :, 0], w1l[:, :, :, 1],
                     b1h, b1l], axis=3)          # [sg,a,b,8,32]
    w1s = np.ascontiguousarray(
        rows.transpose(1, 3, 0, 2, 4).reshape(4, 8, SGS * 128)).astype(bf)

    # coords rows: [ch0, ch1, cl0, cl1, ch0, ch1, 1, 1]
    ch = coords.astype(np.float32).astype(bf)
    clo = (coords.astype(np.float32) - ch.astype(np.float32)).astype(bf)
    crow = np.stack([ch[:, 0], ch[:, 1], clo[:, 0], clo[:, 1],
                     ch[:, 0], ch[:, 1],
                     np.ones(N, bf), np.ones(N, bf)], axis=0)  # [8, N]
    c8 = np.ascontiguousarray(
        np.broadcast_to(crow[None], (4, 8, N))).astype(bf)

    # ---- L2: fp16, partition 32bt+i, free 32a+o (net 4a+bt) ----
    w2c = (w2[sl, :, :, 0] * s).astype(np.float16).reshape(SGS, 4, 4, H, H)
    w2s = np.ascontiguousarray(
        w2c.transpose(2, 3, 0, 1, 4).reshape(4, 32, SGS * 128)).astype(np.float16)

    # ---- L3: fp16 block-diag per (sg, bt) ----
    w3c = w3[sl, :, :, 0].astype(np.float32).reshape(SGS, 4, 4, H, OUT)
    blk = np.zeros((SGS, 4, 4, H, 4, OUT), np.float32)
    for a in range(4):
        blk[:, a, :, :, a, :] = w3c[:, a]
    w3blk = np.ascontiguousarray(
        blk.transpose(1, 3, 0, 2, 4, 5).reshape(4, 32, SGS * 48)).astype(np.float16)

    # ---- smalls: [0..3] b2aug, [4..7] b2sin, [8] b3 ----
    b2c = b2[sl, :, 0].astype(np.float32).reshape(SGS, 4, 4, H)  # [sg,a,b,o]
    b3c = b3[sl, :, 0].astype(np.float32).reshape(SGS, 4, 4, OUT)
    smalls = np.zeros((128, SGS, 11), np.float32)
    p = np.arange(128)
    a_idx, o_idx = p // 32, p % 32
    for bt in range(4):
        # partition 32a+o of psum bank bt -> net 4a+bt
        smalls[:, :, bt] = (b2c[:, a_idx, bt, o_idx] * s + MAG).T
        smalls[:, :, 4 + bt] = (b2c[:, a_idx, bt, o_idx] * np.float32(W0)
                                + np.float32(BIAS1)).T
        if bt in SD_L2:
            smalls[:, :, 9 + SD_L2.index(bt)] = -(b2c[:, a_idx, bt, o_idx] * s).T
    bt_idx, m_idx = p // 32, p % 32
    a3, c3 = m_idx // 3, m_idx % 3
    for pi in range(128):
        if m_idx[pi] < 12:
            smalls[pi, :, 8] = b3c[:, a3[pi], bt_idx[pi], c3[pi]]
    smalls = np.ascontiguousarray(smalls.reshape(128, SGS * 11))

    negI = np.zeros((128, 32), np.float32)
    for b in range(4):
        negI[32 * b:32 * b + 32] = -np.eye(32)

    return {"w1s": w1s, "w2s": w2s, "w3blk": w3blk, "smalls": smalls,
            "coords": c8, "negI": negI.astype(bf)}


def _unshard(res_list):
    outs = []
    for r in res_list:
        o = r["out"].reshape(2, 4, 4, OUT, SGS, NH)   # [h,bt,a,c,sg,p]
        o = o.transpose(4, 2, 1, 0, 5, 3)             # [sg,a,bt,h,p,c]
        outs.append(np.ascontiguousarray(o.reshape(BPC, N, OUT)))
    return np.concatenate(outs, axis=0)


def _run(inputs, trace=False, trace_kwargs=None):
    global _compiled
    if _compiled is None:
        _compiled = _build_module()
    nc = _compiled
    arrs = {k: np.asarray(v, dtype=np.float32) for k, v in inputs.items()}
    in_maps = [_prep_core_inputs(arrs["w1"], arrs["b1"], arrs["w2"], arrs["b2"],
                                 arrs["w3"], arrs["b3"], arrs["coords"], c)
               for c in range(N_CORES)]
    kw = {}
    if trace:
        kw["trace"] = True
        if trace_kwargs:
            kw.update(trace_kwargs)
    res = bass_utils.run_bass_kernel_spmd(nc, in_maps, core_ids=list(range(N_CORES)),
                                          **kw)
    out = _unshard(res.results)
    return out, res


def kernel(**inputs):
    out, _ = _run(inputs, trace=False)
    return out


# revision 33
# speedup vs baseline: 1.3775x; 1.3775x over previous
"""BatchSiren Trainium2 kernel.

B=2048 independent SIREN MLPs (2->32->32->3, sin activations, w0=30),
each evaluated on the same N=1024 coordinate grid.

Strategy (pure data parallel over 8 cores, 256 nets/core):
- 16 supergroups of 16 nets per core; nets packed 16-at-a-time onto the
  128x128 PE array via 32x32 tile_position.
- All matmuls run single-pass (bf16/fp16, 1 cyc/row). Layer 1 uses an
  exact bf16 hi/lo split of coords+weights (K=8) so its products are
  exact in fp32 PSUM accumulation; layers 2/3 use fp16 weights/acts.
- sin(w0*z): weights pre-scaled by w0/2pi so PSUM holds the angle in
  CYCLE units t. Range reduction: DVE copies PSUM+bias+192 to bf16
  (bf16 RNE at ulp=1 yields round(t)+192 exactly), a -I matmul
  accumulates it back into PSUM leaving frac(t)-192, and ACT Sin reads
  PSUM with scale=+2pi, bias=384pi+..., writing fp16 H to SBUF.
- Output: one [128,2048] fp32 tile per (4sg, half), 4 batched DMAs each.
"""
import numpy as np
import ml_dtypes

import concourse.bacc as bacc
import concourse.bass as bass
import concourse.mybir as mybir
import concourse.tile as tile
from concourse import bass_utils

f32 = mybir.dt.float32
f16 = mybir.dt.float16
bf16 = mybir.dt.bfloat16
AF = mybir.ActivationFunctionType
ALU = mybir.AluOpType

W0 = 30.0
PI = float(np.pi)
TWO_PI = float(2 * np.pi)
MAG = 192.0                      # bf16 magic: exact ints in [176, 208]
BIAS1 = float(384.0 * np.pi)     # 2*pi*MAG
SD_L1 = []        # L1 banks whose subtract runs on DVE (else PE -I)
SD_L2 = [3]       # L2 banks whose subtract runs on DVE
N_CORES = 8
B, N, IN, H, OUT = 2048, 1024, 2, 32, 3
BPC = B // N_CORES        # 256 nets per core
SGS = BPC // 16           # 16 supergroups of 16 nets
NH = N // 2               # 512 points per half
GRP = 4                   # supergroups per output tile

_compiled = None


def _build_module():
    nc = bacc.Bacc("TRN2", target_bir_lowering=False, debug=False)

    d_w1 = nc.dram_tensor("w1s", [4, 8, 128 * SGS], bf16, kind="ExternalInput")
    d_w2 = nc.dram_tensor("w2s", [4, 32, 128 * SGS], f16, kind="ExternalInput")
    d_w3 = nc.dram_tensor("w3blk", [4, 32, 48 * SGS], f16, kind="ExternalInput")
    d_sm = nc.dram_tensor("smalls", [128, 11 * SGS], f32, kind="ExternalInput")
    d_c = nc.dram_tensor("coords", [4, 8, N], bf16, kind="ExternalInput")
    d_ni = nc.dram_tensor("negI", [128, 32], bf16, kind="ExternalInput")
    d_out = nc.dram_tensor("out", [2, 48, SGS, NH], f32, kind="ExternalOutput")

    with tile.TileContext(nc, pool_alloc_mode="queue") as tc:
        with tc.tile_pool(name="const", bufs=1) as cp, \
             tc.tile_pool(name="acts", bufs=2) as ap, \
             tc.tile_pool(name="outp", bufs=4) as op_, \
             tc.tile_pool(name="ps", bufs=4, space="PSUM") as psp:

            # ---- persistent constants (sg0-critical tensors first) ----
            w1sb = cp.tile([128, 128 * SGS], bf16, tag="w1")
            for a in range(4):
                nc.sync.dma_start(w1sb[32 * a:32 * a + 8, :], d_w1[a])
            c4 = cp.tile([128, N], bf16, tag="c4")
            nc.vector.memset(c4[:], 0.0)  # pad rows 8-31 of each group
            for a in range(4):
                nc.sync.dma_start(c4[32 * a:32 * a + 8, :], d_c[a])
            negI = cp.tile([128, 32], bf16, tag="ni")
            nc.sync.dma_start(negI[:], d_ni[:])
            smalls = cp.tile([128, 11 * SGS], f32, tag="sm")
            nc.sync.dma_start(smalls[:], d_sm[:])
            w2sb = cp.tile([128, 128 * SGS], f16, tag="w2")
            q = 128 * SGS // 4
            for ck in range(4):
                for b in range(4):
                    nc.sync.dma_start(
                        w2sb[32 * b:32 * b + 32, q * ck:q * ck + q],
                        d_w2[b, :, q * ck:q * ck + q])
            w3sb = cp.tile([128, 48 * SGS], f16, tag="w3")
            for a in range(4):
                nc.sync.dma_start(w3sb[32 * a:32 * a + 32, :], d_w3[a])
            bias1 = cp.tile([128, 1], f32, tag="b1")
            nc.vector.memset(bias1[:], BIAS1)
            mag192 = cp.tile([128, 1], f32, tag="mg")
            nc.vector.memset(mag192[:], MAG)

            # ---- per-(sg,h) stages ----
            def stage_A(sg, h):       # L1 matmuls -> PZ1 (4 banks)
                PZ1 = [psp.tile([128, 512], f32, tag="P1",
                                name=f"pz1_{sg}_{h}_{a}") for a in range(4)]
                for a in range(4):
                    nc.tensor.matmul(
                        out=PZ1[a][:, :],
                        lhsT=w1sb[32 * a:32 * a + 8,
                                  128 * sg:128 * sg + 128],
                        rhs=c4[32 * a:32 * a + 8, NH * h:NH * h + NH],
                        start=True, stop=(a in SD_L1),
                        tile_position=(32 * a, 0))
                return PZ1

            def stage_E1(sg, h, PZ1):  # round-evac L1 -> R1 bf16
                R1 = ap.tile([128, 2048], bf16, tag="R")
                for a in range(4):
                    nc.vector.tensor_scalar(
                        R1[:, 512 * a:512 * a + 512], PZ1[a][:],
                        MAG, 0.0, ALU.add, ALU.add)
                return R1

            def stage_S1(sg, h, PZ1, R1):  # -I accumulate (PE banks only)
                for a in range(4):
                    if a in SD_L1:
                        continue
                    for b in range(4):
                        nc.tensor.matmul(
                            out=PZ1[a][32 * b:32 * b + 32, :],
                            lhsT=negI[32 * b:32 * b + 32, :],
                            rhs=R1[32 * b:32 * b + 32, 512 * a:512 * a + 512],
                            start=False, stop=True,
                            tile_position=(32 * b, 32 * b))
                if not SD_L1:
                    return None
                Q1 = ap.tile([128, 512 * len(SD_L1)], f32, tag="Q1")
                for i, a in enumerate(SD_L1):
                    nc.vector.scalar_tensor_tensor(
                        Q1[:, 512 * i:512 * i + 512],
                        R1[:, 512 * a:512 * a + 512], 0.0, PZ1[a][:],
                        ALU.add, ALU.subtract)
                return Q1

            def stage_N1(sg, h, PZ1, Q1):  # Sin -> H1 fp16
                H1 = ap.tile([128, 2048], f16, tag="H1")
                for a in range(4):
                    if a not in SD_L1:
                        nc.scalar.activation(
                            H1[:, 512 * a:512 * a + 512], PZ1[a][:],
                            AF.Sin, bias=bias1[:], scale=TWO_PI)
                for i, a in enumerate(SD_L1):
                    nc.scalar.activation(
                        H1[:, 512 * a:512 * a + 512],
                        Q1[:, 512 * i:512 * i + 512],
                        AF.Sin, bias=bias1[:], scale=-TWO_PI)
                return H1

            def stage_B(sg, h, H1):    # L2 matmuls -> PZ2 (4 banks)
                PZ2 = [psp.tile([128, 512], f32, tag="P2",
                                name=f"pz2_{sg}_{h}_{bt}") for bt in range(4)]
                for bt in range(4):
                    for a in range(4):
                        nc.tensor.matmul(
                            out=PZ2[bt][32 * a:32 * a + 32, :],
                            lhsT=w2sb[32 * bt:32 * bt + 32,
                                      128 * sg + 32 * a:128 * sg + 32 * a + 32],
                            rhs=H1[32 * bt:32 * bt + 32,
                                   512 * a:512 * a + 512],
                            start=True, stop=(bt in SD_L2),
                            tile_position=(32 * bt, 32 * a))
                return PZ2

            def stage_E2(sg, h, PZ2):  # round-evac L2 (+b2) -> R2 bf16
                R2 = ap.tile([128, 2048], bf16, tag="R")
                nc.scalar.activation(
                    R2[:, 0:512], PZ2[0][:], AF.Identity,
                    bias=smalls[:, 11 * sg:11 * sg + 1], scale=1.0)
                for bt in range(1, 4):
                    nc.vector.tensor_scalar(
                        R2[:, 512 * bt:512 * bt + 512], PZ2[bt][:],
                        smalls[:, 11 * sg + bt:11 * sg + bt + 1], 0.0,
                        ALU.add, ALU.add)
                return R2

            def stage_S2(sg, h, PZ2, R2):
                for bt in range(4):
                    if bt in SD_L2:
                        continue
                    for a in range(4):
                        nc.tensor.matmul(
                            out=PZ2[bt][32 * a:32 * a + 32, :],
                            lhsT=negI[32 * a:32 * a + 32, :],
                            rhs=R2[32 * a:32 * a + 32, 512 * bt:512 * bt + 512],
                            start=False, stop=True,
                            tile_position=(32 * a, 32 * a))
                if not SD_L2:
                    return None
                Q2 = ap.tile([128, 512 * len(SD_L2)], f32, tag="Q2")
                for i, bt in enumerate(SD_L2):
                    nc.vector.scalar_tensor_tensor(
                        Q2[:, 512 * i:512 * i + 512],
                        R2[:, 512 * bt:512 * bt + 512],
                        smalls[:, 11 * sg + 9 + i:11 * sg + 9 + i + 1],
                        PZ2[bt][:], ALU.add, ALU.subtract)
                return Q2

            def stage_N2(sg, h, PZ2, Q2):  # Sin (+b2 bias) -> H2 fp16
                H2 = ap.tile([128, 2048], f16, tag="H2")
                for bt in range(4):
                    if bt not in SD_L2:
                        nc.scalar.activation(
                            H2[:, 512 * bt:512 * bt + 512], PZ2[bt][:],
                            AF.Sin,
                            bias=smalls[:, 11 * sg + 4 + bt:11 * sg + 4 + bt + 1],
                            scale=TWO_PI)
                if False:
                    nc.scalar.activation(
                        H2[:, 1024:2048], Q2[:, 0:1024],
                        AF.Sin, bias=bias1[:], scale=-TWO_PI)
                else:
                    for i, bt in enumerate(SD_L2):
                        nc.scalar.activation(
                            H2[:, 512 * bt:512 * bt + 512],
                            Q2[:, 512 * i:512 * i + 512],
                            AF.Sin, bias=bias1[:], scale=-TWO_PI)
                return H2

            def stage_C(sg, h, H2):    # L3 matmuls -> PC (1 bank)
                PC = psp.tile([128, 512], f32, tag="P1",
                              name=f"pc_{sg}_{h}")
                for bt in range(4):
                    nc.tensor.matmul(
                        out=PC[32 * bt:32 * bt + 12, :],
                        lhsT=w3sb[:, 48 * sg + 12 * bt:48 * sg + 12 * bt + 12],
                        rhs=H2[:, 512 * bt:512 * bt + 512],
                        start=True, stop=True,
                        tile_position=(0, 32 * bt))
                return PC

            def stage_E3(sg, h, PC, OT):  # +b3, evac to output tile
                sgl = sg % GRP
                dst = OT[:, 512 * sgl:512 * sgl + 512]
                bias = smalls[:, 11 * sg + 8:11 * sg + 9]
                if h == 0:
                    nc.vector.tensor_scalar(dst, PC[:], bias, 0.0,
                                            ALU.add, ALU.add)
                else:
                    nc.scalar.activation(dst, PC[:], AF.Identity,
                                         bias=bias, scale=1.0)

            def flush_out(g, h, OT):
                for bt in range(4):
                    nc.sync.dma_start(
                        d_out[h, 12 * bt:12 * bt + 12, GRP * g:GRP * g + GRP, :],
                        OT[32 * bt:32 * bt + 12, :])

            # ---- software-pipelined emission ----
            # V: [E1x4, E2c1..3, E3]  S: [E2c0, N1x4, N2x4, E3]
            # T: [A, S1, S2, B, C]
            OTs = {}

            def get_OT(sg, h):
                g = sg // GRP
                if (g, h) not in OTs:
                    OTs[(g, h)] = op_.tile([128, 512 * GRP], f32, tag="OT",
                                           name=f"ot_{g}_{h}")
                return OTs[(g, h)]

            prev = None
            for sg in range(SGS):
                for h in range(2):
                    PZ1 = stage_A(sg, h)
                    if prev is not None:
                        psg, ph, pPZ2 = prev
                        R2 = ap.tile([128, 2048], bf16, tag="R",
                                     name=f"r2_{psg}_{ph}")
                        nc.scalar.activation(
                            R2[:, 0:512], pPZ2[0][:], AF.Identity,
                            bias=smalls[:, 11 * psg:11 * psg + 1], scale=1.0)
                        nc.scalar.activation(
                            R2[:, 512:1024], pPZ2[1][:], AF.Identity,
                            bias=smalls[:, 11 * psg + 1:11 * psg + 2],
                            scale=1.0)
                        # S2 banks 0,1 -I early: fill PE gap during E1
                        for bt0 in range(2):
                            for a2 in range(4):
                                nc.tensor.matmul(
                                    out=pPZ2[bt0][32 * a2:32 * a2 + 32, :],
                                    lhsT=negI[32 * a2:32 * a2 + 32, :],
                                    rhs=R2[32 * a2:32 * a2 + 32,
                                           512 * bt0:512 * bt0 + 512],
                                    start=False, stop=True,
                                    tile_position=(32 * a2, 32 * a2))
                    R1 = stage_E1(sg, h, PZ1)
                    Q1 = stage_S1(sg, h, PZ1, R1)
                    H1 = stage_N1(sg, h, PZ1, Q1)
                    if prev is not None:
                        Q2p = ap.tile([128, 512 * len(SD_L2)], f32, tag="Q2",
                                      name=f"q2_{psg}_{ph}")
                        for bt in range(2, 4):
                            nc.vector.tensor_scalar(
                                R2[:, 512 * bt:512 * bt + 512], pPZ2[bt][:],
                                smalls[:, 11 * psg + bt:11 * psg + bt + 1],
                                0.0, ALU.add, ALU.add)
                        for bt in range(2, 4):
                            if bt in SD_L2:
                                i = SD_L2.index(bt)
                                nc.vector.scalar_tensor_tensor(
                                    Q2p[:, 512 * i:512 * i + 512],
                                    R2[:, 512 * bt:512 * bt + 512],
                                    smalls[:, 11 * psg + 9 + i:11 * psg + 9 + i + 1],
                                    pPZ2[bt][:], ALU.add, ALU.subtract)
                                continue
                            for a2 in range(4):
                                nc.tensor.matmul(
                                    out=pPZ2[bt][32 * a2:32 * a2 + 32, :],
                                    lhsT=negI[32 * a2:32 * a2 + 32, :],
                                    rhs=R2[32 * a2:32 * a2 + 32,
                                           512 * bt:512 * bt + 512],
                                    start=False, stop=True,
                                    tile_position=(32 * a2, 32 * a2))
                        H2 = stage_N2(psg, ph, pPZ2, Q2p)
                    PZ2 = stage_B(sg, h, H1)
                    if prev is not None:
                        PC = stage_C(psg, ph, H2)
                        OT = get_OT(psg, ph)
                        stage_E3(psg, ph, PC, OT)
                        if (psg % GRP) == GRP - 1:
                            flush_out(psg // GRP, ph, OT)
                    prev = (sg, h, PZ2)
            # epilogue
            psg, ph, pPZ2 = prev
            R2 = stage_E2(psg, ph, pPZ2)
            Q2 = stage_S2(psg, ph, pPZ2, R2)
            H2 = stage_N2(psg, ph, pPZ2, Q2)
            PC = stage_C(psg, ph, H2)
            OT = get_OT(psg, ph)
            stage_E3(psg, ph, PC, OT)
            flush_out(psg // GRP, ph, OT)

    nc.compile()
    return nc


def _prep_core_inputs(w1, b1, w2, b2, w3, b3, coords, core):
    s = np.float32(W0 / TWO_PI)
    B0 = core * BPC
    sl = slice(B0, B0 + BPC)
    bf = ml_dtypes.bfloat16

    # ---- L1: bf16 hi/lo split, K=8 ----
    # net (sg, a, b) = batch B0 + 16sg + 4a + b
    w1c = (w1[sl, :, :, 0] * s).astype(np.float32).reshape(SGS, 4, 4, IN, H)
    b1c = (b1[sl, :, 0] * s).astype(np.float32).reshape(SGS, 4, 4, H)
    w1h = w1c.astype(bf)
    w1l = (w1c - w1h.astype(np.float32)).astype(bf)
    b1h = b1c.astype(bf)
    b1l = (b1c - b1h.astype(np.float32)).astype(bf)
    # rows: [wh0, wh1, wh0, wh1, wl0, wl1, bh, bl]
    rows = np.stack([w1h[:, :, :, 0], w1h[:, :, :, 1],
                     w1h[:, :, :, 0], w1h[:, :, :, 1],
                     w1l[:, :, :, 0], w1l[:, :, :, 1],
                     b1h, b1l], axis=3)          # [sg,a,b,8,32]
    w1s = np.ascontiguousarray(
        rows.transpose(1, 3, 0, 2, 4).reshape(4, 8, SGS * 128)).astype(bf)

    # coords rows: [ch0, ch1, cl0, cl1, ch0, ch1, 1, 1]
    ch = coords.astype(np.float32).astype(bf)
    clo = (coords.astype(np.float32) - ch.astype(np.float32)).astype(bf)
    crow = np.stack([ch[:, 0], ch[:, 1], clo[:, 0], clo[:, 1],
                     ch[:, 0], ch[:, 1],
                     np.ones(N, bf), np.ones(N, bf)], axis=0)  # [8, N]
    c8 = np.ascontiguousarray(
        np.broadcast_to(crow[None], (4, 8, N))).astype(bf)

    # ---- L2: fp16, partition 32bt+i, free 32a+o (net 4a+bt) ----
    w2c = (w2[sl, :, :, 0] * s).astype(np.float16).reshape(SGS, 4, 4, H, H)
    w2s = np.ascontiguousarray(
        w2c.transpose(2, 3, 0, 1, 4).reshape(4, 32, SGS * 128)).astype(np.float16)

    # ---- L3: fp16 block-diag per (sg, bt) ----
    w3c = w3[sl, :, :, 0].astype(np.float32).reshape(SGS, 4, 4, H, OUT)
    blk = np.zeros((SGS, 4, 4, H, 4, OUT), np.float32)
    for a in range(4):
        blk[:, a, :, :, a, :] = w3c[:, a]
    w3blk = np.ascontiguousarray(
        blk.transpose(1, 3, 0, 2, 4, 5).reshape(4, 32, SGS * 48)).astype(np.float16)

    # ---- smalls: [0..3] b2aug, [4..7] b2sin, [8] b3 ----
    b2c = b2[sl, :, 0].astype(np.float32).reshape(SGS, 4, 4, H)  # [sg,a,b,o]
    b3c = b3[sl, :, 0].astype(np.float32).reshape(SGS, 4, 4, OUT)
    smalls = np.zeros((128, SGS, 11), np.float32)
    p = np.arange(128)
    a_idx, o_idx = p // 32, p % 32
    for bt in range(4):
        # partition 32a+o of psum bank bt -> net 4a+bt
        smalls[:, :, bt] = (b2c[:, a_idx, bt, o_idx] * s + MAG).T
        smalls[:, :, 4 + bt] = (b2c[:, a_idx, bt, o_idx] * np.float32(W0)
                                + np.float32(BIAS1)).T
        if bt in SD_L2:
            smalls[:, :, 9 + SD_L2.index(bt)] = -(b2c[:, a_idx, bt, o_idx] * s).T
    bt_idx, m_idx = p // 32, p % 32
    a3, c3 = m_idx // 3, m_idx % 3
    for pi in range(128):
        if m_idx[pi] < 12:
            smalls[pi, :, 8] = b3c[:, a3[pi], bt_idx[pi], c3[pi]]
    smalls = np.ascontiguousarray(smalls.reshape(128, SGS * 11))

    negI = np.zeros((128, 32), np.float32)
    for b in range(4):
        negI[32 * b:32 * b + 32] = -np.eye(32)

    return {"w1s": w1s, "w2s": w2s, "w3blk": w3blk, "smalls": smalls,
            "coords": c8, "negI": negI.astype(bf)}


def _unshard(res_list):
    outs = []
    for r in res_list:
        o = r["out"].reshape(2, 4, 4, OUT, SGS, NH)   # [h,bt,a,c,sg,p]
        o = o.transpose(4, 2, 1, 0, 5, 3)             # [sg,a,bt,h,p,c]
        outs.append(np.ascontiguousarray(o.reshape(BPC, N, OUT)))
    return np.concatenate(outs, axis=0)


def _run(inputs, trace=False, trace_kwargs=None):
    global _compiled
    if _compiled is None:
        _compiled = _build_module()
    nc = _compiled
    arrs = {k: np.asarray(v, dtype=np.float32) for k, v in inputs.items()}
    in_maps = [_prep_core_inputs(arrs["w1"], arrs["b1"], arrs["w2"], arrs["b2"],
                                 arrs["w3"], arrs["b3"], arrs["coords"], c)
               for c in range(N_CORES)]
    kw = {}
    if trace:
        kw["trace"] = True
        if trace_kwargs:
            kw.update(trace_kwargs)
    res = bass_utils.run_bass_kernel_spmd(nc, in_maps, core_ids=list(range(N_CORES)),
                                          **kw)
    out = _unshard(res.results)
    return out, res


def kernel(**inputs):
    out, _ = _run(inputs, trace=False)
    return out
